# revision 52
# baseline (speedup 1.0000x reference)
"""MoE expert-network kernel for 8 Trainium2 NeuronCores.

Strategy: expert parallelism (E == n_cores == 8). The host dispatches each
token to its expert's core (an all-to-all in numpy), folds the inference-mode
BatchNorm into the expert weights/bias, and each core runs one dense
[cap, 512] @ [512, 512] GEMM fused with bias + SiLU via the activation engine.

All device tensors are laid out host-side as the exact SBUF tile images
(128-partition-major, block-contiguous per token tile) so every DMA is a
plain 2D contiguous copy with multi-KB lines.

Per-core device program (identical on all cores, SPMD):
  inputs : xs [128, KC*cap]      fp16 - token tiles, partition-major blocks
           wx [128, MC*KC*128 + KC*128] fp16 - BN-folded weights (m-major
                                        blocks) ++ the first x tile
           bs [128, MC]          fp32 - BN-folded bias tile image
  output : os [128, MC*cap]      fp16 - silu(x @ W + b), (tile, m)-major
x is shipped fp16 (~2e-4 rel error, halves the dominant stream); the host
scatters the result back into the full [B, 512] fp32 output.

Pipeline design notes (from perfetto traces):
  - the weights + first x tile ride ONE DMA at the head of the sync ring
    (a separate scalar-ring weight load gets round-robin-starved to ~25%
    rate by the x burst, stalling every matmul group);
  - tiny lead tiles (128/256/512 tokens) so the x stream stays ahead of
    warm matmul consumption for every tile prefix; a 256-token tail tile
    keeps the final ACT->store chain short;
  - dummy matmuls on a zeroed tile bridge the ~5us data lead-in so the
    PE's HAM clock-gate is warm (2.4GHz) when real work arrives, and a
    dummy SILU pulls the ~1.3us ACT table loads off the critical path;
  - psum tiles span 2 banks: one SILU instruction reads up to 1024
    columns, halving the ACT engine's 352-cycle per-op overhead;
  - stores are per (m-pair, tile) slices fired right after each SILU:
    smooth out-stream without flooding the GpSimd SWDGE queue (~1us
    descriptor emission per store op); the last tile's stores ride the
    scalar HWDGE ring for low completion latency.
"""

import sys

for _p in ("/opt/trn_rl_repo",):
    if _p not in sys.path:
        sys.path.append(_p)

import numpy as np

import concourse.bass as bass
import concourse.mybir as mybir
import concourse.tile as tile
from concourse import bacc
from concourse.bass_utils import run_bass_kernel_spmd

B = 32768
IN = 512
HID = 512
E = 8
NCORES = 8
EPS = 1e-5
P = 128  # SBUF partitions
NT = 512  # matmul moving-dim chunk (one fp32 PSUM bank)

KC = IN // P  # contraction chunks
MC = HID // P  # output-feature chunks
NWARM = 9  # HAM-prewarm dummy matmuls (N=512, cold ~427ns each); they bridge
# the PE from ~1.5us to ~5.4us, when the weights + first x tile have landed
# (x stream ~330 GB/s per-core share), and fully soak the HAM cold window
# so real matmuls start at 2.4GHz.


def plan_sizes(cap: int) -> list:
    """Token-tile sizes: tiny tiles at the start (fast pipeline ramp: first
    matmul can begin after only a 128-token DMA) and a tiny tail tile (short
    final ACT->store chain), 1024-wide tiles in the middle."""
    if cap < 1280:  # not reachable for the real token distribution
        return [min(512, cap - o) for o in range(0, cap, 512)]
    sizes = [128, 256, 512]
    # Reserve a 512-token tail tile: its per-m SILUs (~720ns) are SHORTER
    # than their matmul groups (~852ns), so the ACT engine tracks the PE
    # through the final tile instead of queueing 4 small SILUs after the
    # last matmul (a 256 tail measured ~2us of serial ACT tail).
    rem = cap - 896 - 512
    while rem >= 1024:
        sizes.append(1024)
        rem -= 1024
    if rem:
        sizes.append(rem)
    sizes.append(512)
    return sizes


def build_bass(cap: int, act: str = "silu") -> bass.Bass:
    nc = bacc.Bacc(
        "TRN2",
        target_bir_lowering=False,
        debug=False,
        enable_asserts=False,
        num_devices=NCORES,
    )
    f32 = mybir.dt.float32
    f16 = mybir.dt.float16

    tiles = []
    n0 = 0
    for s in plan_sizes(cap):
        tiles.append((n0, s))
        n0 += s
    s0 = tiles[0][1]
    WOFF = MC * KC * P  # x-tile-1 offset inside the fused wx image

    # wx = weights ++ first x tile: ONE leading DMA so the first matmul
    # group waits on a single completion instead of two serialized ones.
    xs = nc.dram_tensor("xs", [P, KC * cap], f16, kind="ExternalInput").ap()
    wx = nc.dram_tensor("wx", [P, WOFF + KC * s0], f16, kind="ExternalInput").ap()
    bs = nc.dram_tensor("bs", [P, MC], f32, kind="ExternalInput").ap()
    os_ = nc.dram_tensor("os", [P, MC * cap], f16, kind="ExternalOutput").ap()

    with tile.TileContext(nc) as tc:
        with (
            tc.tile_pool(name="wpool", bufs=1) as wpool,
            tc.tile_pool(name="xpool", bufs=6) as xpool,
            tc.tile_pool(name="opool", bufs=3) as opool,
            tc.tile_pool(name="pp", bufs=4, space="PSUM") as pp,
        ):
            # The wx image (weights + first x tile) rides the SAME (sync)
            # ring as the other x tiles, at the head of the FIFO: it
            # streams at the full per-core HBM share, so no matmul ever
            # waits on weights. (A separate scalar-ring weight load gets
            # starved to ~25% rate by the x burst instead.)
            wxt = wpool.tile([P, WOFF + KC * s0], f16, tag="wx", name="wx")
            nc.sync.dma_start(out=wxt, in_=wx)
            bt = wpool.tile([P, MC], f32, tag="bt", name="bt")
            nc.scalar.dma_start(out=bt, in_=bs)

            # Dummy matmuls on a zeroed scratch tile bridge the initial DMA
            # wait: the PE is busy from ~0.5us, so the HAM clock-gate window
            # (~3.4us of sustained activity) is partly paid while the first
            # token tiles are still in flight.
            warm = wpool.tile([P, NT], f16, tag="warm", name="warm")
            nc.gpsimd.memset(warm, 0.0)
            if act == "silu":
                # Tiny dummy SILU: walrus places the ACT table load right
                # before the first activation on each path; doing one now
                # (on the idle ACT engine, during the DMA ramp) keeps the
                # ~1.5us table load off the steady-state critical path.
                sact = wpool.tile([P, 16], f16, tag="sact", name="sact")
                nc.scalar.activation(
                    sact, warm[:, :16], mybir.ActivationFunctionType.Silu
                )
            wps = pp.tile([P, NT], f32, tag="ps", name="wps")
            for _ in range(NWARM):
                nc.tensor.matmul(
                    wps, lhsT=warm[:, :P], rhs=warm, start=True, stop=True
                )

            for ti, (n0, nt) in enumerate(tiles):
                if ti == 0:
                    xt = None  # tile 1 lives inside the fused wx image
                else:
                    xt = xpool.tile([P, KC, nt], f16, tag="xt", name="xt")
                    nc.sync.dma_start(
                        out=xt, in_=xs[:, KC * n0 : KC * (n0 + nt)]
                    )
                ot = opool.tile([P, MC, nt], f16, tag="ot", name="ot")
                ng = -(-nt // NT)  # 512-chunks in this tile (<= 2)
                if ti == len(tiles) - 1 and nt == NT and act == "silu":
                    # Final tile, final m-group split in half: half A's
                    # SILU + store overlap half B's matmuls, and the
                    # terminal store is only 64KB — cuts ~1us of serial
                    # post-last-matmul tail. Separate PSUM tiles per half
                    # (ACT read + PE write of one bank is fatal).
                    for m in range(MC):
                        h = NT // 2 if m == MC - 1 else nt
                        for off0 in range(0, nt, h):
                            ps = pp.tile([P, h], f32, tag="ps", name="ps")
                            for k in range(KC):
                                nc.tensor.matmul(
                                    ps,
                                    lhsT=wxt[
                                        :, (m * KC + k) * P : (m * KC + k + 1) * P
                                    ],
                                    rhs=xt[:, k, off0 : off0 + h],
                                    start=(k == 0),
                                    stop=(k == KC - 1),
                                )
                            nc.scalar.activation(
                                ot[:, m, off0 : off0 + h],
                                ps,
                                mybir.ActivationFunctionType.Silu,
                                bias=bt[:, m : m + 1],
                            )
                            nc.scalar.dma_start(
                                out=os_[
                                    :,
                                    MC * n0 + m * nt + off0 : MC * n0
                                    + m * nt
                                    + off0
                                    + h,
                                ],
                                in_=ot[:, m, off0 : off0 + h],
                            )
                    continue
                for m in range(MC):
                    # ng PSUM banks; one SILU reads the whole [P, nt] span
                    ps = pp.tile([P, ng * NT], f32, tag="ps", name="ps")
                    for g in range(ng):
                        off = g * NT
                        ns = min(NT, nt - off)
                        for k in range(KC):
                            rhs = (
                                wxt[:, WOFF + k * nt + off : WOFF + k * nt + off + ns]
                                if ti == 0
                                else xt[:, k, off : off + ns]
                            )
                            nc.tensor.matmul(
                                ps[:, off : off + ns],
                                lhsT=wxt[
                                    :, (m * KC + k) * P : (m * KC + k + 1) * P
                                ],
                                rhs=rhs,
                                start=(k == 0),
                                stop=(k == KC - 1),
                            )
                    osl = ot[:, m]
                    pview = ps[:, :nt]
                    if act == "silu":
                        nc.scalar.activation(
                            osl,
                            pview,
                            mybir.ActivationFunctionType.Silu,
                            bias=bt[:, m : m + 1],
                        )
                    else:
                        # CoreSim has no Silu: Identity+Sigmoid+mul
                        yt = opool.tile([P, nt], f32, tag="yt", name="yt")
                        nc.scalar.activation(
                            yt,
                            pview,
                            mybir.ActivationFunctionType.Identity,
                            bias=bt[:, m : m + 1],
                        )
                        st = opool.tile([P, nt], f32, tag="st", name="st")
                        nc.scalar.activation(
                            st,
                            pview,
                            mybir.ActivationFunctionType.Sigmoid,
                            bias=bt[:, m : m + 1],
                        )
                        nc.vector.tensor_mul(osl, yt, st)
                    # Store m-pairs (after the m=1 / m=3 SILUs): smooth
                    # out-stream on the (otherwise idle) GpSimd SWDGE ring
                    # without flooding the Q7 descriptor queue (~1us
                    # emission per store op). The last tile stores per-m on
                    # scalar HWDGE: lower completion latency, and the
                    # terminal transfer (the one the exit drain waits on)
                    # is half the size.
                    if ti == len(tiles) - 1:
                        nc.scalar.dma_start(
                            out=os_[:, MC * n0 + m * nt : MC * n0 + (m + 1) * nt],
                            in_=osl,
                        )
                    elif m % 2 == 1:
                        nc.gpsimd.dma_start(
                            out=os_[
                                :, MC * n0 + (m - 1) * nt : MC * n0 + (m + 1) * nt
                            ],
                            in_=ot[:, m - 1 : m + 1],
                        )

    nc.compile()
    return nc


def prepare(inputs: dict) -> tuple:
    x = np.ascontiguousarray(np.asarray(inputs["x"], dtype=np.float32))
    idx = np.asarray(inputs["expert_indices"]).astype(np.int64)
    ew = np.asarray(inputs["expert_weights"], dtype=np.float32)
    eb = np.asarray(inputs["expert_biases"], dtype=np.float32)
    gw = np.asarray(inputs["bn_weights"], dtype=np.float32)
    gb = np.asarray(inputs["bn_biases"], dtype=np.float32)
    rm = np.asarray(inputs["running_mean"], dtype=np.float32)
    rv = np.asarray(inputs["running_var"], dtype=np.float32)

    # Fold inference BN into the expert weight/bias:
    #   y = (x @ W + eb - rm) * gw/sqrt(rv+eps) + gb = x @ (W*s) + (eb-rm)*s + gb
    s = gw / np.sqrt(rv + EPS)
    wf = ew * s[:, None, :]
    bf = (eb - rm) * s + gb

    perms = [np.nonzero(idx == e)[0] for e in range(E)]
    counts = [len(p) for p in perms]
    cap = max(512, -(-max(counts) // P) * P)
    tiles = []
    n0 = 0
    for t in plan_sizes(cap):
        tiles.append((n0, t))
        n0 += t

    in_maps = []
    for e in range(E):
        xT = np.zeros((IN, cap), dtype=np.float16)
        if counts[e]:
            xT[:, : counts[e]] = x[perms[e]].T.astype(np.float16)
        xv = xT.reshape(KC, P, cap)
        xs = np.empty((P, KC * cap), dtype=np.float16)
        for n0, nt in tiles:
            xs[:, KC * n0 : KC * (n0 + nt)] = (
                xv[:, :, n0 : n0 + nt].transpose(1, 0, 2).reshape(P, KC * nt)
            )
        # m-major weight image: ws[p, ((m*KC + k)*P + j)] = W[k*P + p, m*P + j]
        ws = (
            wf[e]
            .astype(np.float16)
            .reshape(KC, P, MC, P)
            .transpose(1, 2, 0, 3)
            .reshape(P, MC * KC * P)
        )
        # fused leading image: weights ++ first x tile
        s0 = tiles[0][1]
        wx = np.concatenate([ws, xs[:, : KC * s0]], axis=1)
        bs = np.ascontiguousarray(bf[e].reshape(MC, P).T)
        in_maps.append({"xs": xs, "wx": np.ascontiguousarray(wx), "bs": bs})
    return cap, tiles, perms, counts, in_maps


def combine(results: list, cap, tiles, perms, counts) -> np.ndarray:
    out = np.empty((B, HID), dtype=np.float32)
    for e in range(E):
        if not counts[e]:
            continue
        ob = results[e]["os"]
        oT = np.empty((HID, cap), dtype=np.float32)
        for n0, nt in tiles:
            # per-(tile, m) blocks: [P, nt] at column MC*n0 + m*nt
            oT[:, n0 : n0 + nt] = (
                ob[:, MC * n0 : MC * (n0 + nt)]
                .reshape(P, MC, nt)
                .transpose(1, 0, 2)
                .reshape(HID, nt)
            )
        out[perms[e]] = oT[:, : counts[e]].T
    return out


def kernel(**inputs) -> np.ndarray:
    cap, tiles, perms, counts, in_maps = prepare(inputs)
    nc = build_bass(cap)
    res = run_bass_kernel_spmd(nc, in_maps, core_ids=list(range(NCORES)))
    return combine(res.results, cap, tiles, perms, counts)


# revision 54
# speedup vs baseline: 1.0354x; 1.0354x over previous
"""MoE expert-network kernel for 8 Trainium2 NeuronCores.

Strategy: expert parallelism (E == n_cores == 8). The host dispatches each
token to its expert's core (an all-to-all in numpy), folds the inference-mode
BatchNorm into the expert weights/bias, and each core runs one dense
[cap, 512] @ [512, 512] GEMM fused with bias + SiLU via the activation engine.

All device tensors are laid out host-side as the exact SBUF tile images
(128-partition-major, block-contiguous per token tile) so every DMA is a
plain 2D contiguous copy with multi-KB lines.

Per-core device program (identical on all cores, SPMD):
  inputs : xs [128, KC*cap]      fp16 - token tiles, partition-major blocks
           wx [128, MC*KC*128 + KC*128] fp16 - BN-folded weights (m-major
                                        blocks) ++ the first x tile
           bs [128, MC]          fp32 - BN-folded bias tile image
  output : os [128, MC*cap]      fp16 - silu(x @ W + b), (tile, m)-major
x is shipped fp16 (~2e-4 rel error, halves the dominant stream); the host
scatters the result back into the full [B, 512] fp32 output.

Pipeline design notes (from perfetto traces):
  - the weights + first x tile ride ONE DMA at the head of the sync ring
    (a separate scalar-ring weight load gets round-robin-starved to ~25%
    rate by the x burst, stalling every matmul group);
  - tiny lead tiles (128/256/512 tokens) so the x stream stays ahead of
    warm matmul consumption for every tile prefix; a 256-token tail tile
    keeps the final ACT->store chain short;
  - dummy matmuls on a zeroed tile bridge the ~5us data lead-in so the
    PE's HAM clock-gate is warm (2.4GHz) when real work arrives, and a
    dummy SILU pulls the ~1.3us ACT table loads off the critical path;
  - psum tiles span 2 banks: one SILU instruction reads up to 1024
    columns, halving the ACT engine's 352-cycle per-op overhead;
  - stores are per (m-pair, tile) slices fired right after each SILU:
    smooth out-stream without flooding the GpSimd SWDGE queue (~1us
    descriptor emission per store op); the last tile's stores ride the
    scalar HWDGE ring for low completion latency.
"""

import sys

for _p in ("/opt/trn_rl_repo",):
    if _p not in sys.path:
        sys.path.append(_p)

import numpy as np

import concourse.bass as bass
import concourse.mybir as mybir
import concourse.tile as tile
from concourse import bacc
from concourse.bass_utils import run_bass_kernel_spmd

B = 32768
IN = 512
HID = 512
E = 8
NCORES = 8
EPS = 1e-5
P = 128  # SBUF partitions
NT = 512  # matmul moving-dim chunk (one fp32 PSUM bank)

KC = IN // P  # contraction chunks
MC = HID // P  # output-feature chunks
NWARM = 9  # HAM-prewarm dummy matmuls (N=512, cold ~427ns each); they bridge
# the PE from ~1.5us to ~5.4us, when the weights + first x tile have landed
# (x stream ~330 GB/s per-core share), and fully soak the HAM cold window
# so real matmuls start at 2.4GHz.


def plan_sizes(cap: int) -> list:
    """Token-tile sizes: tiny tiles at the start (fast pipeline ramp: first
    matmul can begin after only a 128-token DMA) and a tiny tail tile (short
    final ACT->store chain), 1024-wide tiles in the middle."""
    if cap < 1280:  # not reachable for the real token distribution
        return [min(512, cap - o) for o in range(0, cap, 512)]
    sizes = [128, 256, 512]
    # Reserve a 512-token tail tile: its per-m SILUs (~720ns) are SHORTER
    # than their matmul groups (~852ns), so the ACT engine tracks the PE
    # through the final tile instead of queueing 4 small SILUs after the
    # last matmul (a 256 tail measured ~2us of serial ACT tail).
    rem = cap - 896 - 512
    while rem >= 1024:
        sizes.append(1024)
        rem -= 1024
    if rem:
        sizes.append(rem)
    sizes.append(512)
    return sizes


def build_bass(cap: int, act: str = "silu") -> bass.Bass:
    nc = bacc.Bacc(
        "TRN2",
        target_bir_lowering=False,
        debug=False,
        enable_asserts=False,
        num_devices=NCORES,
    )
    f32 = mybir.dt.float32
    f16 = mybir.dt.float16

    tiles = []
    n0 = 0
    for s in plan_sizes(cap):
        tiles.append((n0, s))
        n0 += s
    s0 = tiles[0][1]
    WOFF = MC * KC * P  # x-tile-1 offset inside the fused wx image

    # wx = weights ++ first x tile: ONE leading DMA so the first matmul
    # group waits on a single completion instead of two serialized ones.
    xs = nc.dram_tensor("xs", [P, KC * cap], f16, kind="ExternalInput").ap()
    wx = nc.dram_tensor("wx", [P, WOFF + KC * s0], f16, kind="ExternalInput").ap()
    bs = nc.dram_tensor("bs", [P, MC], f32, kind="ExternalInput").ap()
    os_ = nc.dram_tensor("os", [P, MC * cap], f16, kind="ExternalOutput").ap()

    with tile.TileContext(nc) as tc:
        with (
            tc.tile_pool(name="wpool", bufs=1) as wpool,
            tc.tile_pool(name="xpool", bufs=6) as xpool,
            tc.tile_pool(name="opool", bufs=3) as opool,
            tc.tile_pool(name="pp", bufs=4, space="PSUM") as pp,
        ):
            # The wx image (weights + first x tile) rides the SAME (sync)
            # ring as the other x tiles, at the head of the FIFO: it
            # streams at the full per-core HBM share, so no matmul ever
            # waits on weights. (A separate scalar-ring weight load gets
            # starved to ~25% rate by the x burst instead.)
            wxt = wpool.tile([P, WOFF + KC * s0], f16, tag="wx", name="wx")
            nc.sync.dma_start(out=wxt, in_=wx)
            bt = wpool.tile([P, MC], f32, tag="bt", name="bt")
            nc.scalar.dma_start(out=bt, in_=bs)

            # Dummy matmuls on a zeroed scratch tile bridge the initial DMA
            # wait: the PE is busy from ~0.5us, so the HAM clock-gate window
            # (~3.4us of sustained activity) is partly paid while the first
            # token tiles are still in flight.
            warm = wpool.tile([P, NT], f16, tag="warm", name="warm")
            nc.gpsimd.memset(warm, 0.0)
            if act == "silu":
                # Tiny dummy SILU: walrus places the ACT table load right
                # before the first activation on each path; doing one now
                # (on the idle ACT engine, during the DMA ramp) keeps the
                # ~1.5us table load off the steady-state critical path.
                sact = wpool.tile([P, 16], f16, tag="sact", name="sact")
                nc.scalar.activation(
                    sact, warm[:, :16], mybir.ActivationFunctionType.Silu
                )
            wps = pp.tile([P, NT], f32, tag="ps", name="wps")
            for _ in range(NWARM):
                nc.tensor.matmul(
                    wps, lhsT=warm[:, :P], rhs=warm, start=True, stop=True
                )

            for ti, (n0, nt) in enumerate(tiles):
                if ti == 0:
                    xt = None  # tile 1 lives inside the fused wx image
                else:
                    xt = xpool.tile([P, KC, nt], f16, tag="xt", name="xt")
                    nc.sync.dma_start(
                        out=xt, in_=xs[:, KC * n0 : KC * (n0 + nt)]
                    )
                ot = opool.tile([P, MC, nt], f16, tag="ot", name="ot")
                ng = -(-nt // NT)  # 512-chunks in this tile (<= 2)
                for m in range(MC):
                    # ng PSUM banks; one SILU reads the whole [P, nt] span
                    ps = pp.tile([P, ng * NT], f32, tag="ps", name="ps")
                    for g in range(ng):
                        off = g * NT
                        ns = min(NT, nt - off)
                        for k in range(KC):
                            rhs = (
                                wxt[:, WOFF + k * nt + off : WOFF + k * nt + off + ns]
                                if ti == 0
                                else xt[:, k, off : off + ns]
                            )
                            nc.tensor.matmul(
                                ps[:, off : off + ns],
                                lhsT=wxt[
                                    :, (m * KC + k) * P : (m * KC + k + 1) * P
                                ],
                                rhs=rhs,
                                start=(k == 0),
                                stop=(k == KC - 1),
                            )
                    osl = ot[:, m]
                    pview = ps[:, :nt]
                    if act == "silu":
                        nc.scalar.activation(
                            osl,
                            pview,
                            mybir.ActivationFunctionType.Silu,
                            bias=bt[:, m : m + 1],
                        )
                    else:
                        # CoreSim has no Silu: Identity+Sigmoid+mul
                        yt = opool.tile([P, nt], f32, tag="yt", name="yt")
                        nc.scalar.activation(
                            yt,
                            pview,
                            mybir.ActivationFunctionType.Identity,
                            bias=bt[:, m : m + 1],
                        )
                        st = opool.tile([P, nt], f32, tag="st", name="st")
                        nc.scalar.activation(
                            st,
                            pview,
                            mybir.ActivationFunctionType.Sigmoid,
                            bias=bt[:, m : m + 1],
                        )
                        nc.vector.tensor_mul(osl, yt, st)
                    # Store m-pairs (after the m=1 / m=3 SILUs): smooth
                    # out-stream on the (otherwise idle) GpSimd SWDGE ring
                    # without flooding the Q7 descriptor queue (~1us
                    # emission per store op). The last tile stores per-m on
                    # scalar HWDGE: lower completion latency, and the
                    # terminal transfer (the one the exit drain waits on)
                    # is half the size.
                    if ti == len(tiles) - 1:
                        # NOT on the scalar ring: a HWDGE store trigger
                        # costs ~850ns on the issuing sequencer, and on
                        # ACT those triggers interleave with (and delay)
                        # the final SILUs. GpSimd (idle Q7) takes the
                        # non-terminal stores; the terminal one rides the
                        # idle sync HWDGE ring for low completion latency.
                        out_eng = nc.sync if m == MC - 1 else nc.gpsimd
                        out_eng.dma_start(
                            out=os_[:, MC * n0 + m * nt : MC * n0 + (m + 1) * nt],
                            in_=osl,
                        )
                    elif m % 2 == 1:
                        nc.gpsimd.dma_start(
                            out=os_[
                                :, MC * n0 + (m - 1) * nt : MC * n0 + (m + 1) * nt
                            ],
                            in_=ot[:, m - 1 : m + 1],
                        )

    nc.compile()
    return nc


def prepare(inputs: dict) -> tuple:
    x = np.ascontiguousarray(np.asarray(inputs["x"], dtype=np.float32))
    idx = np.asarray(inputs["expert_indices"]).astype(np.int64)
    ew = np.asarray(inputs["expert_weights"], dtype=np.float32)
    eb = np.asarray(inputs["expert_biases"], dtype=np.float32)
    gw = np.asarray(inputs["bn_weights"], dtype=np.float32)
    gb = np.asarray(inputs["bn_biases"], dtype=np.float32)
    rm = np.asarray(inputs["running_mean"], dtype=np.float32)
    rv = np.asarray(inputs["running_var"], dtype=np.float32)

    # Fold inference BN into the expert weight/bias:
    #   y = (x @ W + eb - rm) * gw/sqrt(rv+eps) + gb = x @ (W*s) + (eb-rm)*s + gb
    s = gw / np.sqrt(rv + EPS)
    wf = ew * s[:, None, :]
    bf = (eb - rm) * s + gb

    perms = [np.nonzero(idx == e)[0] for e in range(E)]
    counts = [len(p) for p in perms]
    cap = max(512, -(-max(counts) // P) * P)
    tiles = []
    n0 = 0
    for t in plan_sizes(cap):
        tiles.append((n0, t))
        n0 += t

    in_maps = []
    for e in range(E):
        xT = np.zeros((IN, cap), dtype=np.float16)
        if counts[e]:
            xT[:, : counts[e]] = x[perms[e]].T.astype(np.float16)
        xv = xT.reshape(KC, P, cap)
        xs = np.empty((P, KC * cap), dtype=np.float16)
        for n0, nt in tiles:
            xs[:, KC * n0 : KC * (n0 + nt)] = (
                xv[:, :, n0 : n0 + nt].transpose(1, 0, 2).reshape(P, KC * nt)
            )
        # m-major weight image: ws[p, ((m*KC + k)*P + j)] = W[k*P + p, m*P + j]
        ws = (
            wf[e]
            .astype(np.float16)
            .reshape(KC, P, MC, P)
            .transpose(1, 2, 0, 3)
            .reshape(P, MC * KC * P)
        )
        # fused leading image: weights ++ first x tile
        s0 = tiles[0][1]
        wx = np.concatenate([ws, xs[:, : KC * s0]], axis=1)
        bs = np.ascontiguousarray(bf[e].reshape(MC, P).T)
        in_maps.append({"xs": xs, "wx": np.ascontiguousarray(wx), "bs": bs})
    return cap, tiles, perms, counts, in_maps


def combine(results: list, cap, tiles, perms, counts) -> np.ndarray:
    out = np.empty((B, HID), dtype=np.float32)
    for e in range(E):
        if not counts[e]:
            continue
        ob = results[e]["os"]
        oT = np.empty((HID, cap), dtype=np.float32)
        for n0, nt in tiles:
            # per-(tile, m) blocks: [P, nt] at column MC*n0 + m*nt
            oT[:, n0 : n0 + nt] = (
                ob[:, MC * n0 : MC * (n0 + nt)]
                .reshape(P, MC, nt)
                .transpose(1, 0, 2)
                .reshape(HID, nt)
            )
        out[perms[e]] = oT[:, : counts[e]].T
    return out


def kernel(**inputs) -> np.ndarray:
    cap, tiles, perms, counts, in_maps = prepare(inputs)
    nc = build_bass(cap)
    res = run_bass_kernel_spmd(nc, in_maps, core_ids=list(range(NCORES)))
    return combine(res.results, cap, tiles, perms, counts)


# revision 55
# speedup vs baseline: 1.0727x; 1.0361x over previous
"""MoE expert-network kernel for 8 Trainium2 NeuronCores.

Strategy: expert parallelism (E == n_cores == 8). The host dispatches each
token to its expert's core (an all-to-all in numpy), folds the inference-mode
BatchNorm into the expert weights/bias, and each core runs one dense
[cap, 512] @ [512, 512] GEMM fused with bias + SiLU via the activation engine.

All device tensors are laid out host-side as the exact SBUF tile images
(128-partition-major, block-contiguous per token tile) so every DMA is a
plain 2D contiguous copy with multi-KB lines.

Per-core device program (identical on all cores, SPMD):
  inputs : xs [128, KC*cap]      fp16 - token tiles, partition-major blocks
           wx [128, MC*KC*128 + KC*128] fp16 - BN-folded weights (m-major
                                        blocks) ++ the first x tile
           bs [128, MC]          fp32 - BN-folded bias tile image
  output : os [128, MC*cap]      fp16 - silu(x @ W + b), (tile, m)-major
x is shipped fp16 (~2e-4 rel error, halves the dominant stream); the host
scatters the result back into the full [B, 512] fp32 output.

Pipeline design notes (from perfetto traces):
  - the weights + first x tile ride ONE DMA at the head of the sync ring
    (a separate scalar-ring weight load gets round-robin-starved to ~25%
    rate by the x burst, stalling every matmul group);
  - tiny lead tiles (128/256/512 tokens) so the x stream stays ahead of
    warm matmul consumption for every tile prefix; a 256-token tail tile
    keeps the final ACT->store chain short;
  - dummy matmuls on a zeroed tile bridge the ~5us data lead-in so the
    PE's HAM clock-gate is warm (2.4GHz) when real work arrives, and a
    dummy SILU pulls the ~1.3us ACT table loads off the critical path;
  - psum tiles span 2 banks: one SILU instruction reads up to 1024
    columns, halving the ACT engine's 352-cycle per-op overhead;
  - stores are per (m-pair, tile) slices fired right after each SILU:
    smooth out-stream without flooding the GpSimd SWDGE queue (~1us
    descriptor emission per store op); the last tile's stores ride the
    scalar HWDGE ring for low completion latency.
"""

import sys

for _p in ("/opt/trn_rl_repo",):
    if _p not in sys.path:
        sys.path.append(_p)

import numpy as np

import concourse.bass as bass
import concourse.mybir as mybir
import concourse.tile as tile
from concourse import bacc
from concourse.bass_utils import run_bass_kernel_spmd

B = 32768
IN = 512
HID = 512
E = 8
NCORES = 8
EPS = 1e-5
P = 128  # SBUF partitions
NT = 512  # matmul moving-dim chunk (one fp32 PSUM bank)

KC = IN // P  # contraction chunks
MC = HID // P  # output-feature chunks
NWARM = 9  # HAM-prewarm dummy matmuls (N=512, cold ~427ns each); they bridge
# the PE from ~1.5us to ~5.4us, when the weights + first x tile have landed
# (x stream ~330 GB/s per-core share), and fully soak the HAM cold window
# so real matmuls start at 2.4GHz.


def plan_sizes(cap: int) -> list:
    """Token-tile sizes: tiny tiles at the start (fast pipeline ramp: first
    matmul can begin after only a 128-token DMA) and a tiny tail tile (short
    final ACT->store chain), 1024-wide tiles in the middle."""
    if cap < 1280:  # not reachable for the real token distribution
        return [min(512, cap - o) for o in range(0, cap, 512)]
    sizes = [128, 256, 512]
    # Reserve a 512-token tail tile: its per-m SILUs (~720ns) are SHORTER
    # than their matmul groups (~852ns), so the ACT engine tracks the PE
    # through the final tile instead of queueing 4 small SILUs after the
    # last matmul (a 256 tail measured ~2us of serial ACT tail).
    rem = cap - 896 - 512
    while rem >= 1024:
        sizes.append(1024)
        rem -= 1024
    if rem:
        sizes.append(rem)
    sizes.append(512)
    return sizes


def build_bass(cap: int, act: str = "silu") -> bass.Bass:
    nc = bacc.Bacc(
        "TRN2",
        target_bir_lowering=False,
        debug=False,
        enable_asserts=False,
        num_devices=NCORES,
    )
    f32 = mybir.dt.float32
    f16 = mybir.dt.float16

    tiles = []
    n0 = 0
    for s in plan_sizes(cap):
        tiles.append((n0, s))
        n0 += s
    s0 = tiles[0][1]
    WOFF = MC * KC * P  # x-tile-1 offset inside the fused wx image

    # wx = weights ++ first x tile: ONE leading DMA so the first matmul
    # group waits on a single completion instead of two serialized ones.
    xs = nc.dram_tensor("xs", [P, KC * cap], f16, kind="ExternalInput").ap()
    wx = nc.dram_tensor("wx", [P, WOFF + KC * s0], f16, kind="ExternalInput").ap()
    bs = nc.dram_tensor("bs", [P, MC], f32, kind="ExternalInput").ap()
    os_ = nc.dram_tensor("os", [P, MC * cap], f16, kind="ExternalOutput").ap()

    with tile.TileContext(nc) as tc:
        with (
            tc.tile_pool(name="wpool", bufs=1) as wpool,
            tc.tile_pool(name="xpool", bufs=6) as xpool,
            tc.tile_pool(name="opool", bufs=3) as opool,
            tc.tile_pool(name="pp", bufs=4, space="PSUM") as pp,
        ):
            # The wx image (weights + first x tile) rides the SAME (sync)
            # ring as the other x tiles, at the head of the FIFO: it
            # streams at the full per-core HBM share, so no matmul ever
            # waits on weights. (A separate scalar-ring weight load gets
            # starved to ~25% rate by the x burst instead.)
            wxt = wpool.tile([P, WOFF + KC * s0], f16, tag="wx", name="wx")
            nc.sync.dma_start(out=wxt, in_=wx)
            bt = wpool.tile([P, MC], f32, tag="bt", name="bt")
            nc.scalar.dma_start(out=bt, in_=bs)

            # Dummy matmuls on a zeroed scratch tile bridge the initial DMA
            # wait: the PE is busy from ~0.5us, so the HAM clock-gate window
            # (~3.4us of sustained activity) is partly paid while the first
            # token tiles are still in flight.
            warm = wpool.tile([P, NT], f16, tag="warm", name="warm")
            nc.gpsimd.memset(warm, 0.0)
            if act == "silu":
                # Tiny dummy SILU: walrus places the ACT table load right
                # before the first activation on each path; doing one now
                # (on the idle ACT engine, during the DMA ramp) keeps the
                # ~1.5us table load off the steady-state critical path.
                sact = wpool.tile([P, 16], f16, tag="sact", name="sact")
                nc.scalar.activation(
                    sact, warm[:, :16], mybir.ActivationFunctionType.Silu
                )
            wps = pp.tile([P, NT], f32, tag="ps", name="wps")
            for _ in range(NWARM):
                nc.tensor.matmul(
                    wps, lhsT=warm[:, :P], rhs=warm, start=True, stop=True
                )

            for ti, (n0, nt) in enumerate(tiles):
                if ti == 0:
                    xt = None  # tile 1 lives inside the fused wx image
                else:
                    xt = xpool.tile([P, KC, nt], f16, tag="xt", name="xt")
                    nc.sync.dma_start(
                        out=xt, in_=xs[:, KC * n0 : KC * (n0 + nt)]
                    )
                ot = opool.tile([P, MC, nt], f16, tag="ot", name="ot")
                ng = -(-nt // NT)  # 512-chunks in this tile (<= 2)
                for m in range(MC):
                    # ng PSUM banks; one SILU reads the whole [P, nt] span
                    ps = pp.tile([P, ng * NT], f32, tag="ps", name="ps")
                    for g in range(ng):
                        off = g * NT
                        ns = min(NT, nt - off)
                        for k in range(KC):
                            rhs = (
                                wxt[:, WOFF + k * nt + off : WOFF + k * nt + off + ns]
                                if ti == 0
                                else xt[:, k, off : off + ns]
                            )
                            nc.tensor.matmul(
                                ps[:, off : off + ns],
                                lhsT=wxt[
                                    :, (m * KC + k) * P : (m * KC + k + 1) * P
                                ],
                                rhs=rhs,
                                start=(k == 0),
                                stop=(k == KC - 1),
                            )
                    osl = ot[:, m]
                    pview = ps[:, :nt]
                    if act == "silu":
                        nc.scalar.activation(
                            osl,
                            pview,
                            mybir.ActivationFunctionType.Silu,
                            bias=bt[:, m : m + 1],
                        )
                    else:
                        # CoreSim has no Silu: Identity+Sigmoid+mul
                        yt = opool.tile([P, nt], f32, tag="yt", name="yt")
                        nc.scalar.activation(
                            yt,
                            pview,
                            mybir.ActivationFunctionType.Identity,
                            bias=bt[:, m : m + 1],
                        )
                        st = opool.tile([P, nt], f32, tag="st", name="st")
                        nc.scalar.activation(
                            st,
                            pview,
                            mybir.ActivationFunctionType.Sigmoid,
                            bias=bt[:, m : m + 1],
                        )
                        nc.vector.tensor_mul(osl, yt, st)
                    # Store m-pairs (after the m=1 / m=3 SILUs): smooth
                    # out-stream on the (otherwise idle) GpSimd SWDGE ring
                    # without flooding the Q7 descriptor queue (~1us
                    # emission per store op). The last tile stores per-m on
                    # scalar HWDGE: lower completion latency, and the
                    # terminal transfer (the one the exit drain waits on)
                    # is half the size.
                    if ti == len(tiles) - 1:
                        nc.scalar.dma_start(
                            out=os_[:, MC * n0 + m * nt : MC * n0 + (m + 1) * nt],
                            in_=osl,
                        )
                    elif m % 2 == 1:
                        nc.gpsimd.dma_start(
                            out=os_[
                                :, MC * n0 + (m - 1) * nt : MC * n0 + (m + 1) * nt
                            ],
                            in_=ot[:, m - 1 : m + 1],
                        )

    nc.compile()
    return nc


def prepare(inputs: dict) -> tuple:
    x = np.ascontiguousarray(np.asarray(inputs["x"], dtype=np.float32))
    idx = np.asarray(inputs["expert_indices"]).astype(np.int64)
    ew = np.asarray(inputs["expert_weights"], dtype=np.float32)
    eb = np.asarray(inputs["expert_biases"], dtype=np.float32)
    gw = np.asarray(inputs["bn_weights"], dtype=np.float32)
    gb = np.asarray(inputs["bn_biases"], dtype=np.float32)
    rm = np.asarray(inputs["running_mean"], dtype=np.float32)
    rv = np.asarray(inputs["running_var"], dtype=np.float32)

    # Fold inference BN into the expert weight/bias:
    #   y = (x @ W + eb - rm) * gw/sqrt(rv+eps) + gb = x @ (W*s) + (eb-rm)*s + gb
    s = gw / np.sqrt(rv + EPS)
    wf = ew * s[:, None, :]
    bf = (eb - rm) * s + gb

    perms = [np.nonzero(idx == e)[0] for e in range(E)]
    counts = [len(p) for p in perms]
    cap = max(512, -(-max(counts) // P) * P)
    tiles = []
    n0 = 0
    for t in plan_sizes(cap):
        tiles.append((n0, t))
        n0 += t

    in_maps = []
    for e in range(E):
        xT = np.zeros((IN, cap), dtype=np.float16)
        if counts[e]:
            xT[:, : counts[e]] = x[perms[e]].T.astype(np.float16)
        xv = xT.reshape(KC, P, cap)
        xs = np.empty((P, KC * cap), dtype=np.float16)
        for n0, nt in tiles:
            xs[:, KC * n0 : KC * (n0 + nt)] = (
                xv[:, :, n0 : n0 + nt].transpose(1, 0, 2).reshape(P, KC * nt)
            )
        # m-major weight image: ws[p, ((m*KC + k)*P + j)] = W[k*P + p, m*P + j]
        ws = (
            wf[e]
            .astype(np.float16)
            .reshape(KC, P, MC, P)
            .transpose(1, 2, 0, 3)
            .reshape(P, MC * KC * P)
        )
        # fused leading image: weights ++ first x tile
        s0 = tiles[0][1]
        wx = np.concatenate([ws, xs[:, : KC * s0]], axis=1)
        bs = np.ascontiguousarray(bf[e].reshape(MC, P).T)
        in_maps.append({"xs": xs, "wx": np.ascontiguousarray(wx), "bs": bs})
    return cap, tiles, perms, counts, in_maps


def combine(results: list, cap, tiles, perms, counts) -> np.ndarray:
    out = np.empty((B, HID), dtype=np.float32)
    for e in range(E):
        if not counts[e]:
            continue
        ob = results[e]["os"]
        oT = np.empty((HID, cap), dtype=np.float32)
        for n0, nt in tiles:
            # per-(tile, m) blocks: [P, nt] at column MC*n0 + m*nt
            oT[:, n0 : n0 + nt] = (
                ob[:, MC * n0 : MC * (n0 + nt)]
                .reshape(P, MC, nt)
                .transpose(1, 0, 2)
                .reshape(HID, nt)
            )
        out[perms[e]] = oT[:, : counts[e]].T
    return out


def kernel(**inputs) -> np.ndarray:
    cap, tiles, perms, counts, in_maps = prepare(inputs)
    nc = build_bass(cap)
    res = run_bass_kernel_spmd(nc, in_maps, core_ids=list(range(NCORES)))
    return combine(res.results, cap, tiles, perms, counts)
